# revision 31
# baseline (speedup 1.0000x reference)
"""Trainium2 Bass kernel for nn_CNF_76355928588411.

Data-parallel over N across 8 NeuronCores. The tiny t-conditioned hypernet
(three dense layers -> W, U, gate, B; depends only on the scalar t) is
evaluated once on the host in fp32; its ~50KB of derived weights are
replicated to all cores. The N-compute (h = tanh(x@W^T + B), dx = h^T@U/E,
Jacobian-trace column) runs on the devices.

Layout: windows of 1024 samples packed as [128, 512] tiles - two sample
groups (s=0,1) stacked on the partition dim, so every matmul uses the full
128x128 PE array via block-diagonal weights:
  mm1: hp = blockdiag(W^T, W^T) @ xw          [128, 512] psum (512 fp16 rows)
  ACT: t1 = tanh(hp + [B;B])                  [128, 512] fp16
  DVE: t2 = t1*t1 (2x fp16 mode, per pair)    [128, 1024] fp16
  mm2: po = blockdiag(U/E, U/E)^T @ t1 -> dx  [128ch, 512] psum
  mm3: dl = [wu|0 ; 0|wu]^T @ t2              [2, 512] psum (raw sum wu*h^2)
  ACT+DVE: po f32 -> ob fp16 (224/288 col split, one instr per 2 windows)
  DMA: ob -> dxh fp16; dl -> dlh f32 (host applies (dl - sum wu)/E)

All I/O is fp16 except the tiny dl column (f32). GPSIMD cannot touch PSUM
and DMA cannot read PSUM, so the dx egress (512 cols/window) must share
ACT+DVE with tanh/square - that egress is the ~885ns/window critical
resource; DMA (~853ns/window incl. dl garbage rows) and PE (~640) sit just
under it. dl matmuls for groups of 3 windows write one [66, 512] psum tile
at partition bases {0,32,64}; one DVE copy stages the group to SBUF and a
single [66, 512] DMA (4 dead row-pairs) emits it. po/t1/t2 tiles span 2
windows so the psum access bubbles amortize; PSUM = 2+2*2+2 = 8 banks.
The mm2/dl/egress stage trails mm1/tanh by 3 windows so the PE's in-order
queue never stalls on the tanh->square chain; the first x fetch is split so
window 0 starts ~2.5us earlier; the last dx batch DMAs per-pair to shrink
the drain.
"""

import sys

sys.path.insert(0, "/opt/trn_rl_repo")

import numpy as np

import concourse.bass as bass
from concourse import bacc
import concourse.mybir as mybir
import concourse.tile as tile
from concourse.bass_utils import run_bass_kernel_spmd

F32 = mybir.dt.float32
F16 = mybir.dt.float16
AF = mybir.ActivationFunctionType

E, D, H_DIM, N = 64, 64, 512, 262144
BLOCK = E * D
NCORES = 8
NSH = N // NCORES          # 32768 samples per core
WIN = 1024                 # samples per window ([128, 512] dual-packed)
NWIN = NSH // WIN          # 32 windows
WQ = 4                     # windows per x/dx DMA batch
NQ = NWIN // WQ            # 8 DMA batches
CA = 224                   # dx egress cols per window copied by ACT
SQP = 192                  # square cols per window computed by Pool engine
DLG = 3                    # windows per dl psum group tile
NDLG = (NWIN + DLG - 1) // DLG   # 11 dl groups
SKEW = 3                   # iterations between mm1 and mm2 stages

_CACHED = {}


def _build_nc():
    nc = bacc.Bacc("TRN2", target_bir_lowering=False, debug=False,
                   num_devices=NCORES)
    xt = nc.dram_tensor("xt", [128, NSH // 2], F16, kind="ExternalInput")
    # cst cols: 0:128 Wblk, 128:256 UPblk, 256:258 wublk, 258:260 B (f32 bits)
    cst = nc.dram_tensor("cst", [128, 260], F16, kind="ExternalInput")
    dxh = nc.dram_tensor("dxh", [128, NSH // 2], F16, kind="ExternalOutput")
    dlh = nc.dram_tensor("dlh", [66, 512 * NDLG], F16, kind="ExternalOutput")

    with tile.TileContext(nc) as tc:
        with (
            tc.tile_pool(name="consts", bufs=1) as consts,
            tc.tile_pool(name="xin", bufs=4) as xin,
            tc.tile_pool(name="t1p", bufs=3) as t1p,
            tc.tile_pool(name="t2p", bufs=3) as t2p,
            tc.tile_pool(name="dlsp", bufs=2) as dlsp,
            tc.tile_pool(name="obp", bufs=2) as obp,
            tc.tile_pool(name="ps_h", bufs=2, space="PSUM") as ps_h,
            tc.tile_pool(name="ps_o", bufs=2, space="PSUM") as ps_o,
            tc.tile_pool(name="ps_dl", bufs=2, space="PSUM") as ps_dl,
        ):
            cst_t = consts.tile([128, 260], F16)
            xqs = {}

            def fetch(q, split=False):
                xq_t = xin.tile([128, WQ * 512], F16, tag="xq")
                xqs[q] = xq_t
                lo = q * WQ * 512
                if split:
                    nc.sync.dma_start(out=xq_t[:, 0:512],
                                      in_=xt[:, lo:lo + 512])
                    nc.sync.dma_start(out=xq_t[:, 512:1024],
                                      in_=xt[:, lo + 512:lo + 1024])
                    nc.sync.dma_start(out=xq_t[:, 1024:WQ * 512],
                                      in_=xt[:, lo + 1024:lo + WQ * 512])
                else:
                    nc.sync.dma_start(out=xq_t, in_=xt[:, lo:lo + WQ * 512])

            # warm the ACT table at t=0 (hoists the 1.3us table load) and
            # start the PE pstate ramp clock with a tiny matmul
            nc.scalar.dma_start(out=cst_t, in_=cst[:, :])
            dummy = consts.tile([128, 2], F32)
            nc.vector.memset(dummy, 0.0)
            nc.scalar.activation(dummy[0:1, 1:2], dummy[0:1, 0:1], AF.Tanh)
            dummy16 = consts.tile([128, 2], F16)
            nc.vector.memset(dummy16, 0.0)
            warm_ps = ps_h.tile([128, 512], F32, name="warm_ps", tag="hp")
            nc.tensor.matmul(warm_ps[0:2, 0:2], dummy16, dummy16[:, 0:2],
                             start=True, stop=True)
            fetch(0, split=True)
            fetch(1)
            fetch(2)

            wblk = cst_t[:, 0:128]
            upblk = cst_t[:, 128:256]
            wublk = cst_t[:, 256:258]
            bdup_t = cst_t[:, 258:260].bitcast(F32)

            t1_cur = {}    # pair index -> t1 tile / (t1, t2)
            po_cur = {}    # current 2-window po tile
            dl_cur = {}    # current dl group psum tile
            ob_cur = {}    # current ob batch tile

            def stage_a(w):
                # window w: x fetch, mm1, tanh; square per completed pair
                if w % WQ == 1 and w // WQ + 3 < NQ:
                    fetch(w // WQ + 3)
                xq = xqs[w // WQ]
                xw = xq[:, (w % WQ) * 512:(w % WQ + 1) * 512]
                hp = ps_h.tile([128, 512], F32, name="hp", tag="hp")
                nc.tensor.matmul(hp, wblk, xw, start=True, stop=True)
                if w % 2 == 0:
                    t1_cur[w // 2] = t1p.tile([128, 1024], F16, name="t1")
                t1 = t1_cur[w // 2]
                if isinstance(t1, tuple):
                    t1 = t1[0]
                half = (w % 2) * 512
                nc.scalar.activation(t1[:, half:half + 512], hp, AF.Tanh,
                                     bias=bdup_t, scale=1.0)
                if w == NWIN - 2:
                    t2 = t2p.tile([128, 1024], F16, name="t2")
                    nc.vector.tensor_mul(t2[:, 0:512], t1[:, 0:512],
                                         t1[:, 0:512])
                    t1_cur[w // 2] = (t1, t2, "half")
                elif w % 2 == 1:
                    prev = t1_cur[w // 2]
                    if isinstance(prev, tuple) and len(prev) == 3:
                        t1, t2 = prev[0], prev[1]
                        nc.vector.tensor_mul(t2[:, 512:1024], t1[:, 512:1024],
                                             t1[:, 512:1024])
                    else:
                        t2 = t2p.tile([128, 1024], F16, name="t2")
                        nc.vector.tensor_mul(t2, t1, t1)
                    t1_cur[w // 2] = (t1, t2)
                if w % WQ == 3:
                    del xqs[w // WQ]

            def stage_b(v):
                # window v (= w - SKEW): mm2, dl matmul, egress, out DMAs
                t1, t2 = t1_cur[v // 2]
                if v % 2 == 0:
                    po_cur[0] = ps_o.tile([128, 1024], F32, name="po")
                po = po_cur[0]
                half = (v % 2) * 512
                nc.tensor.matmul(po[:, half:half + 512], upblk,
                                 t1[:, half:half + 512], start=True, stop=True)
                g, k = v // DLG, v % DLG
                if k == 0:
                    dl_cur[0] = ps_dl.tile([66, 512], F32, name="dl")
                dl = dl_cur[0]
                nc.tensor.matmul(dl[32 * k:32 * k + 2, :], wublk,
                                 t2[:, half:half + 512], start=True, stop=True)
                last_batch = v // WQ == NQ - 1
                if v % WQ == 0:
                    ob_cur[0] = obp.tile([128, WQ * 512], F16, name="ob")
                ob = ob_cur[0]
                if last_batch:
                    # endgame: per-window egress on ACT only (no tanh left on
                    # its queue), per-window dx DMA on alternating queues
                    if v % 2 == 1:
                        del t1_cur[v // 2]
                    q = v % WQ
                    nc.scalar.copy(ob[:, q * 512:(q + 1) * 512],
                                   po[:, half:half + 512])
                    eng = nc.sync if v % 2 == 0 else nc.scalar
                    eng.dma_start(out=dxh[:, v * 512:(v + 1) * 512],
                                  in_=ob[:, q * 512:(q + 1) * 512])
                elif v % 2 == 1:
                    del t1_cur[v // 2]
                    # dx egress for the completed pair (windows v-1, v)
                    pr = ((v - 1) % WQ) // 2
                    po_v = po.rearrange("p (h c) -> p h c", h=2)
                    ob_v = ob.rearrange("p (q c) -> p q c", q=WQ)[
                        :, 2 * pr:2 * pr + 2]
                    nc.scalar.copy(ob_v[:, :, 0:CA], po_v[:, :, 0:CA])
                    nc.vector.tensor_copy(ob_v[:, :, CA:512],
                                          po_v[:, :, CA:512])
                if k == DLG - 1 or v == NWIN - 1:
                    dls = dlsp.tile([66, 512], F16, name="dls", tag="dls")
                    nc.vector.tensor_copy(dls, dl)
                    eng = nc.scalar if v == NWIN - 1 else nc.gpsimd
                    eng.dma_start(
                        out=dlh[:, g * 512:(g + 1) * 512], in_=dls)
                if v % WQ == WQ - 1 and not last_batch:
                    lo = (v // WQ) * WQ * 512
                    nc.sync.dma_start(out=dxh[:, lo:lo + WQ * 512], in_=ob)

            for it in range(NWIN + SKEW):
                # stage B first: its inputs are older, so ready work is
                # never queued behind not-yet-ready stage-A work on any
                # engine's in-order queue
                if it - SKEW >= 0:
                    stage_b(it - SKEW)
                if it < NWIN:
                    stage_a(it)
    nc.compile()
    return nc


def _hypernet(t, W1, b1, W2, b2, W3, b3):
    p = np.tanh(t.reshape(1, 1) @ W1 + b1)
    p = np.tanh(p @ W2 + b2)
    p = (p @ W3 + b3).reshape(-1).astype(np.float32)
    W = p[:BLOCK].reshape(E, D)
    U = p[BLOCK:2 * BLOCK].reshape(E, D)
    G = 1.0 / (1.0 + np.exp(-p[2 * BLOCK:3 * BLOCK].reshape(E, D)))
    U = (U * G).astype(np.float32)
    B = p[3 * BLOCK:].reshape(E, 1).astype(np.float32)
    return W.astype(np.float32), U, B


def kernel(t, x, W1, b1, W2, b2, W3, b3):
    W, U, B = _hypernet(
        np.asarray(t, np.float32), np.asarray(W1, np.float32),
        np.asarray(b1, np.float32), np.asarray(W2, np.float32),
        np.asarray(b2, np.float32), np.asarray(W3, np.float32),
        np.asarray(b3, np.float32),
    )
    wu = np.sum(W * U, axis=1).astype(np.float32)      # [E]

    cst = np.zeros((128, 258), np.float32)
    cst[0:64, 0:64] = W.T
    cst[64:128, 64:128] = W.T
    cst[0:64, 128:192] = U / E
    cst[64:128, 192:256] = U / E
    cst[0:64, 256] = wu
    cst[64:128, 257] = wu
    cst = cst.astype(np.float16)
    bdup = np.concatenate([B, B], axis=0).reshape(128, 1).astype(np.float32)
    cst = np.concatenate([cst, bdup.view(np.float16).reshape(128, 2)], axis=1)

    # x [N, D] -> per-core [128, NSH//2] fp16; sample (c, w, s, j) at
    # partition s*64+d, column w*512+j
    xs = np.asarray(x, np.float16).reshape(NCORES, NWIN, 2, 512, D)
    xs = np.ascontiguousarray(xs.transpose(0, 2, 4, 1, 3))
    xl = xs.reshape(NCORES, 128, NSH // 2)

    if "nc" not in _CACHED:
        _CACHED["nc"] = _build_nc()
    nc = _CACHED["nc"]

    in_maps = [
        {"xt": xl[c], "cst": cst}
        for c in range(NCORES)
    ]
    res = run_bass_kernel_spmd(nc, in_maps, core_ids=list(range(NCORES)))

    out = np.empty((N, D + 1), np.float32)
    od = out[:, :D].reshape(NCORES, NWIN, 2, 512, D)
    ol = out[:, D].reshape(NCORES, NWIN, 2, 512)
    sw = float(np.sum(wu))
    for c in range(NCORES):
        dxc = res.results[c]["dxh"].astype(np.float32)
        od[c] = dxc.reshape(2, D, NWIN, 512).transpose(2, 0, 3, 1)
        dlc = res.results[c]["dlh"].astype(np.float32)
        # window w = g*DLG + k lives at rows 32k:32k+2, group col block g
        dlg = dlc.reshape(33, 2, NDLG, 512)[::16]      # [k, s, g, j]
        dlw = dlg.transpose(2, 0, 1, 3).reshape(NDLG * DLG, 2, 512)[:NWIN]
        ol[c] = (dlw - sw) / E
    return out
